# revision 63
# baseline (speedup 1.0000x reference)
"""GAT 2-layer encoder kernel for Trainium2 (8 NeuronCores, Bass/Tile) — v3.

Strategy (graph/data parallel, dst-sharded), evolved from v2:
  - Layer-1 node table computed LOCALLY on every core from the full x input
    (x is replicated anyway) — no layer-1 AllGather at all. Builds are
    batched (one xT load + one table store DMA per 8 windows).
  - One-hot scatter matrices (ST) generated ON-CHIP with a single DVE
    is_equal per window (offs bf16 vs iota row) — no 41 MB of one-hot DMA.
  - Self-loops ride as a final per-window "self chunk": the window's own
    table rows are loaded contiguously (cheap) and copied into the gather
    buffer; no random gather for the 50k self edges.
  - Trailing gather pad indices are -1 (SWDGE skips them — no fetch).
    ex/exh live in separate tiles so stale pad data stays bounded; G pool
    buffers are memset once at start so uninitialized SBUF can't inject NaN
    (PE computes 0*NaN=NaN in the segment-sum matmul otherwise).
  - Table rows: [h bf16 x256 | al_src f32 x4 | al_dst f32 x4] = 544 B used
    of a 768 B stride (SWDGE elems must be 256 B multiples); writes and the
    layer-2 AllGather move only the used 544 B (strided APs).
  - Layer-2 table still AllGathered (2 pieces, piece-major) and overlapped
    with the edge phase; lo/hi gather split lets layer-N gathers start as
    soon as the first table piece exists.
"""

import math
import os
import sys

import numpy as np

sys.path.insert(0, "/opt/trn_rl_repo")

P = 128          # partitions
TS2 = 384        # bf16 table row stride (768 B = 3*256)
WCOLS = 264      # bf16 cols actually used per row: [h x256|al_src x4|al_dst x4]
D1 = 256
NPIECES = 2


class Cfg:
    def __init__(self, n_nodes=50000, in_dim=128, heads=4, hid=64, n_cores=8):
        self.n_nodes = n_nodes
        self.in_dim = in_dim
        self.heads = heads
        self.hid = hid
        self.n_cores = n_cores
        self.d1 = heads * hid                       # 256
        assert n_nodes % n_cores == 0
        self.shard = n_nodes // n_cores             # 6250
        self.nw = math.ceil(self.shard / P)         # 49
        # piece boundaries in windows (pipelining granularity); skewed so the
        # last AllGather piece (exposed between layers) is small. Piece-0
        # full rows must stay < 32768 (int16 gather indices).
        self.piece_wins = [31, self.nw - 31]
        self.piece_w0 = np.concatenate([[0], np.cumsum(self.piece_wins)])
        # piece row ranges within a shard
        self.piece_r0 = [min(int(self.piece_w0[p]) * P, self.shard)
                         for p in range(NPIECES + 1)]
        self.piece_rows = [self.piece_r0[p + 1] - self.piece_r0[p]
                           for p in range(NPIECES)]
        # full-table piece base rows (piece-major: [p][core][row])
        self.full_pbase = np.concatenate(
            [[0], np.cumsum([n_cores * r for r in self.piece_rows])])
        self.full_piece_rows = [n_cores * r for r in self.piece_rows]

    def win_piece(self, w):
        for p in range(NPIECES):
            if self.piece_w0[p] <= w < self.piece_w0[p + 1]:
                return p
        raise AssertionError(w)

    def full_row(self, node):
        """Global node id -> full-table row index (piece-major layout)."""
        c = node // self.shard
        r = node - c * self.shard
        p = np.searchsorted(self.piece_r0, r, side="right") - 1
        p = np.minimum(p, NPIECES - 1)
        r0 = np.asarray(self.piece_r0)[p]
        prows = np.asarray(self.piece_rows)[p]
        return np.asarray(self.full_pbase)[p] + c * prows + (r - r0)


def _balance_nodes(cfg, edge_index):
    """Permute dst nodes within each (core, piece) block so per-window
    lo/hi in-edge counts are near-uniform (cuts the max-over-core chunk
    count). Returns pnode: node id -> permuted pseudo-node id. Piece
    membership is preserved, so edge lo/hi classes are invariant."""
    import heapq
    NC, SH, N, NW = cfg.n_cores, cfg.shard, cfg.n_nodes, cfg.nw
    src = np.asarray(edge_index[0], dtype=np.int64)
    dst = np.asarray(edge_index[1], dtype=np.int64)
    r1 = cfg.piece_r0[1]
    lo_src = (src % SH) < r1  # piece of src (within its own core block)
    dlo = np.zeros(N, dtype=np.int64)
    dhi = np.zeros(N, dtype=np.int64)
    np.add.at(dlo, dst[lo_src], 1)
    np.add.at(dhi, dst[~lo_src], 1)
    deg = dlo + dhi
    pnode = np.empty(N, dtype=np.int64)
    order_all = np.argsort(-deg, kind="stable")
    core_of = order_all // SH
    r_of = order_all % SH
    piece_of = (r_of >= r1).astype(np.int64)
    for c in range(NC):
        for p in range(NPIECES):
            nodes = order_all[(core_of == c) & (piece_of == p)]
            w0, w1 = int(cfg.piece_w0[p]), int(cfg.piece_w0[p + 1])
            # balance lo and hi in-degree sums jointly (normalized max)
            sl = 1.0 / max(dlo[nodes].sum() / (w1 - w0), 1.0)
            sh_ = 1.0 / max(dhi[nodes].sum() / (w1 - w0), 1.0)
            heap = []
            lo_w = {}
            hi_w = {}
            fill = {}
            for w in range(w0, w1):
                cap = min(P, SH - w * P)
                heap.append((0.0, w, cap))
                fill[w] = 0
                lo_w[w] = 0
                hi_w[w] = 0
            heapq.heapify(heap)
            for n in nodes:
                while True:
                    _, w, cap = heapq.heappop(heap)
                    if fill[w] < cap:
                        break
                pnode[n] = c * SH + w * P + fill[w]
                fill[w] += 1
                lo_w[w] += dlo[n]
                hi_w[w] += dhi[n]
                if fill[w] < cap:
                    pri = max(lo_w[w] * sl, hi_w[w] * sh_)
                    heapq.heappush(heap, (pri, w, cap))
    return pnode


def _plan_edges(cfg, edge_index, pnode):
    """Host-side: per-core, per-window padded edge lists in gather layout.

    Real edges only (self loops handled separately on-chip). Per window the
    chunk order is [lo gathered | hi gathered | self]. Pad slots get gather
    idx -1 (skipped by SWDGE) and dst-offset 255 (dead one-hot column).
    Node ids are pre-permuted through pnode (load balancing).
    """
    NC, SH, NW = cfg.n_cores, cfg.shard, cfg.nw
    src = pnode[np.asarray(edge_index[0], dtype=np.int64)]
    dst = pnode[np.asarray(edge_index[1], dtype=np.int64)]

    frow = cfg.full_row(src)          # table row of each edge's source
    split = int(cfg.full_pbase[1])    # piece boundary (<32768: int16-safe)
    assert split < 32768 and cfg.n_nodes - split < 32768
    core = dst // SH
    win = (dst - core * SH) // P

    order = np.lexsort((frow, win, core))
    frow_s, dst_s, core_s, win_s = frow[order], dst[order], core[order], win[order]
    key = core_s * NW + win_s
    starts = np.searchsorted(key, np.arange(NC * NW))
    ends = np.searchsorted(key, np.arange(NC * NW) + 1)

    lo_edges = [[None] * NW for _ in range(NC)]
    hi_edges = [[None] * NW for _ in range(NC)]
    for c in range(NC):
        for w in range(NW):
            s, e = starts[c * NW + w], ends[c * NW + w]
            es, ed = frow_s[s:e], dst_s[s:e]
            lo = es < split
            lo_edges[c][w] = (es[lo], ed[lo])
            hi_edges[c][w] = (es[~lo], ed[~lo])

    nch_lo = [0] * NW
    nch_hi = [0] * NW
    for w in range(NW):
        ml = max(len(lo_edges[c][w][0]) for c in range(NC))
        mh = max(len(hi_edges[c][w][0]) for c in range(NC))
        nch_lo[w] = math.ceil(ml / P) if ml else 0
        nch_hi[w] = math.ceil(mh / P) if mh else 0

    nch_g = [nch_lo[w] + nch_hi[w] for w in range(NW)]   # gathered chunks
    nch = [g + 1 for g in nch_g]                         # + self chunk
    nch_tot = sum(nch)
    ncols = 8 * sum(nch_g)
    maxck = int(os.environ.get("GAT_MAXCK", "8"))
    # call order (shared with build_program): per window, lo calls then hi
    ncalls = sum(math.ceil(nch_lo[w] / maxck) + math.ceil(nch_hi[w] / maxck)
                 for w in range(NW))

    def wrap16(vals, n_idx):
        cols = n_idx // 16
        out = np.zeros((16, cols), dtype=np.int16)
        out[np.arange(n_idx) % 16, np.arange(n_idx) // 16] = np.asarray(
            vals, dtype=np.int64)
        return np.tile(out, (8, 1))

    import ml_dtypes
    bf = ml_dtypes.bfloat16
    pad0 = os.environ.get("GAT_PAD0") == "1"
    per_core = []
    for c in range(NC):
        gidx = np.zeros((P, ncols), dtype=np.int16)
        offs = np.full((P, nch_tot), 255.0, dtype=np.float32)
        counts = np.zeros((1, max(ncalls, 1)), dtype=np.int32)
        gcol = 0
        ccol = 0
        call = 0
        for w in range(NW):
            rows = min(P, SH - w * P)
            for (es, ed), nchunks, base in (
                (lo_edges[c][w], nch_lo[w], 0),
                (hi_edges[c][w], nch_hi[w], split),
            ):
                if nchunks == 0:
                    continue
                n_idx = nchunks * P
                g = np.full(n_idx, 0 if pad0 else -1, dtype=np.int64)
                k = len(es)
                g[:k] = es - base
                d_local = ed - c * SH
                o = (d_local - w * P).astype(np.float32)
                offs[np.arange(k) % P, ccol + np.arange(k) // P] = o
                # per-call valid counts; zero-count calls fetch one dummy row
                for c0 in range(0, nchunks, maxck):
                    cn = min(maxck, nchunks - c0)
                    kc = min(max(k - c0 * P, 0), cn * P)
                    if kc == 0 and not pad0:
                        g[c0 * P] = 0
                        kc = 1
                    counts[0, call] = cn * P if pad0 else kc
                    call += 1
                gidx[:, gcol:gcol + 8 * nchunks] = wrap16(g, n_idx)
                gcol += 8 * nchunks
                ccol += nchunks
            # self chunk: identity offsets for the window's own rows
            offs[:rows, ccol] = np.arange(rows, dtype=np.float32)
            ccol += 1
        assert gcol == ncols and ccol == nch_tot and call == ncalls
        # duplicate offs columns x2 so the device is_equal can use an
        # innermost [1,2] access (enables the DVE 2x perf mode)
        offs2 = np.repeat(offs, 2, axis=1)
        per_core.append(dict(gidx=gidx, offs=offs2.astype(bf), counts=counts))

    plan = dict(nch_lo=nch_lo, nch_hi=nch_hi, nch=nch, nch_tot=nch_tot,
                ncols=ncols, nch_max=max(nch), ncalls=ncalls, maxck=maxck)
    return plan, per_core


def _ilv(cfg):
    """Head-interleave permutation: new col 4c+h <- old col h*64+c."""
    H, C = cfg.heads, cfg.hid
    p = np.empty(cfg.d1, dtype=np.int64)
    for h in range(H):
        for c in range(C):
            p[c * H + h] = h * C + c
    return p


def _pack_wext(cfg, W, a_src, a_dst, row_ilv=False):
    """[K, 256] weight -> [K, 264] f32: [W | W@Asrc | W@Adst], with the
    256 h columns head-interleaved (col 4c+h). row_ilv also interleaves
    the K rows (for W2, whose input act is interleaved)."""
    K = W.shape[0]
    H, C = cfg.heads, cfg.hid
    out = np.zeros((K, cfg.d1 + 8), dtype=np.float32)
    p = _ilv(cfg)
    out[:, :cfg.d1] = W[:, p]
    for h in range(H):
        out[:, cfg.d1 + h] = W[:, h * C:(h + 1) * C] @ a_src[h]
        out[:, cfg.d1 + 4 + h] = W[:, h * C:(h + 1) * C] @ a_dst[h]
    if row_ilv:
        out = out[p, :]
    return out


def _ap(t, offset_elems, free_pattern):
    import concourse.bass as bass
    return bass.AP(t.tensor, t.offset + offset_elems,
                   [list(t.ap[0])] + [list(p) for p in free_pattern])


def _apf(t, bf_offset, bf_pattern):
    """f32 bitcast view of a bf16 SBUF tile (offsets/strides in bf16 elems)."""
    import concourse.bass as bass
    import concourse.mybir as mybir
    ap = bass.AP(t.tensor, t.offset + bf_offset,
                 [list(t.ap[0])] + [list(p) for p in bf_pattern])
    return ap.bitcast(mybir.dt.float32)


def _apd(t, offset_elems, pattern):
    import concourse.bass as bass
    return bass.AP(t.tensor, t.offset + offset_elems,
                   [list(p) for p in pattern])


def build_program(cfg, plan):
    import concourse.bass as bass
    import concourse.mybir as mybir
    import concourse.tile as tile
    from concourse import bacc
    from concourse.masks import make_identity
    from contextlib import ExitStack

    f32 = mybir.dt.float32
    bf16 = mybir.dt.bfloat16
    i16 = mybir.dt.int16
    H, C = cfg.heads, cfg.hid
    SH, NW, NC = cfg.shard, cfg.nw, cfg.n_cores
    NCH, NCHL, NCHH = plan["nch"], plan["nch_lo"], plan["nch_hi"]
    NCOLS = plan["ncols"]
    NCHMAX = plan["nch_max"]
    N = cfg.n_nodes
    k2_tiles = D1 // P  # 2
    SPLIT = int(cfg.full_pbase[1])

    MAXCK = int(os.environ.get("GAT_MAXCK", "8"))  # <=1024 idx per call
    NQ = int(os.environ.get("GAT_NQ", "4"))        # SWDGE queues (ucode max 4)
    PRE = int(os.environ.get("GAT_PRE", "8"))
    BG = int(os.environ.get("GAT_BG", "4"))         # build windows per DMA
    # collectives require contiguous outputs -> AG moves full 768 B rows
    AGC = TS2
    qctr = [0]

    def next_q():
        q = qctr[0] % NQ
        qctr[0] += 1
        return q

    SCRATCH = int(os.environ.get("GAT_SCRATCH", "65536"))
    nc = bacc.Bacc(num_swdge_queues=NQ, dynamic_dma_scratch_size=SCRATCH)

    xTf = nc.dram_tensor("xTf", [cfg.in_dim, N], bf16, kind="ExternalInput")
    xTs = nc.dram_tensor("xTs", [cfg.in_dim, SH], bf16, kind="ExternalInput")
    w1e = nc.dram_tensor("w1e", [cfg.in_dim, D1 + 8], bf16,
                         kind="ExternalInput")
    w2e = nc.dram_tensor("w2e", [D1, D1 + 8], bf16, kind="ExternalInput")
    gidx_d = nc.dram_tensor("gidx", [P, NCOLS], i16, kind="ExternalInput")
    offs_d = nc.dram_tensor("offs", [P, 2 * plan["nch_tot"]], bf16,
                            kind="ExternalInput")
    cnts_d = nc.dram_tensor("counts", [1, max(plan["ncalls"], 1)],
                            mybir.dt.int32, kind="ExternalInput")
    out_d = nc.dram_tensor("out", [SH, D1], f32, kind="ExternalOutput")

    with ExitStack() as ctx:
        tc = ctx.enter_context(tile.TileContext(nc))
        const = ctx.enter_context(tc.tile_pool(name="const", bufs=1))
        sb = ctx.enter_context(tc.tile_pool(name="sb", bufs=4))
        eps = ctx.enter_context(tc.tile_pool(name="eps", bufs=PRE + 1))
        wk = ctx.enter_context(tc.tile_pool(name="wk", bufs=2))
        psum = ctx.enter_context(tc.tile_pool(name="psum", bufs=2, space="PSUM"))
        dram = ctx.enter_context(tc.tile_pool(name="dram", bufs=1, space="DRAM"))

        # ---- constants
        w1e_sb = const.tile([cfg.in_dim, D1 + 8], bf16)
        nc.sync.dma_start(out=w1e_sb[:], in_=w1e[:, :])
        w2e_sb = [const.tile([P, D1 + 8], bf16, tag=f"w2e{k}",
                             name=f"w2e_sb{k}") for k in range(k2_tiles)]
        for k in range(k2_tiles):
            nc.sync.dma_start(out=w2e_sb[k][:], in_=w2e[k * P:(k + 1) * P, :])
        gidx_sb = const.tile([P, NCOLS], i16)
        nc.sync.dma_start(out=gidx_sb[:], in_=gidx_d[:, :])
        offs_sb = const.tile([P, 2 * plan["nch_tot"]], bf16, tag="offs",
                             name="offs_sb")
        nc.sync.dma_start(out=offs_sb[:], in_=offs_d[:, :])
        cnts_sb = const.tile([1, max(plan["ncalls"], 1)], mybir.dt.int32,
                             tag="cnts", name="cnts_sb")
        nc.sync.dma_start(out=cnts_sb[:], in_=cnts_d[:, :])
        ident = const.tile([P, P], f32)
        make_identity(nc, ident[:])
        ident_bf = const.tile([P, P], bf16, tag="identbf", name="ident_bf")
        make_identity(nc, ident_bf[:])
        iota_bf = const.tile([P, P], bf16, tag="iotabf", name="iota_bf")
        nc.gpsimd.iota(iota_bf[:], pattern=[[1, P]], base=0,
                       channel_multiplier=0,
                       allow_small_or_imprecise_dtypes=True)

        # ---- DRAM tables
        # t1 pieces: local full layer-1 table (piece-major), built locally.
        t1p = [dram.tile([cfg.full_piece_rows[p], TS2], bf16,
                         tag=f"t1p{p}", name=f"t1p{p}") for p in range(NPIECES)]
        # layer-2: local shard pieces (AG input, tight) + shared AG output.
        ts2 = [dram.tile([cfg.piece_rows[p], AGC], bf16,
                         tag=f"ts2_{p}", name=f"ts2_{p}") for p in range(NPIECES)]
        tf2 = [dram.tile([cfg.full_piece_rows[p], TS2], bf16,
                         tag=f"tf2_{p}", name=f"tf2_{p}", addr_space="Shared")
               for p in range(NPIECES)]
        # layer-1 local shard rows (self/al_dst source), tight.
        ts1 = dram.tile([SH, WCOLS], bf16, tag="ts1", name="ts1")
        groups = [list(range(NC))]

        # ---- init G pool buffers (uninitialized SBUF -> NaN via PE 0*NaN)
        for _ in range(PRE + 1):
            g0 = eps.tile([P, NCHMAX * TS2], bf16, tag="G", name="g_init")
            nc.vector.memset(g0[:], 0.0)

        # ---- table build helpers ------------------------------------------
        def build_windows(src_xT, col0, nwin, ncols, store):
            """nwin windows (ncols total cols) from src_xT[:, col0...];
            tsb holds [128, nwin*WCOLS] bf16 (window k at k*WCOLS)."""
            xt = sb.tile([P, BG * P], bf16, tag="xt")
            nc.sync.dma_start(out=xt[:, :ncols],
                              in_=src_xT[:, col0:col0 + ncols])
            tsb = sb.tile([P, BG * WCOLS], bf16, tag="btsb")
            for k in range(nwin):
                kr = min(P, ncols - k * P)
                ps = psum.tile([P, D1 + 8], f32, tag="t2p")
                nc.tensor.matmul(out=ps[:kr, :],
                                 lhsT=xt[:, k * P:k * P + kr],
                                 rhs=w1e_sb[:], start=True, stop=True)
                # row = [h | al_src | al_dst] = ps[:, :264] verbatim (bf16)
                nc.scalar.copy(out=tsb[:kr, k * WCOLS:k * WCOLS + 132],
                               in_=ps[:kr, :132])
                nc.vector.tensor_copy(
                    out=tsb[:kr, k * WCOLS + 132:k * WCOLS + WCOLS],
                    in_=ps[:kr, 132:WCOLS])
            store(tsb, nwin)

        def build_phase(src_xT, src_col0, ncols_total, dst, dst_stride):
            """Build ncols_total/128 windows from src_xT[:, src_col0...] into
            dst (row-major, stride in bf16 elems; writes WCOLS per row)."""
            nwin_t = math.ceil(ncols_total / P)
            w0 = 0
            while w0 < nwin_t:
                nwin = min(BG, nwin_t - w0)
                # trailing partial window handled separately
                full = nwin if (w0 + nwin) * P <= ncols_total else nwin - 1

                def store(tsb, nwin, w0=w0, full=full):
                    if full > 0:
                        nc.sync.dma_start(
                            out=_apd(dst, w0 * P * dst_stride,
                                     [[dst_stride, P],
                                      [P * dst_stride, full], [1, WCOLS]]),
                            in_=_ap(tsb, 0, [[WCOLS, full], [1, WCOLS]]))
                    if full < nwin:
                        rows = ncols_total - (w0 + full) * P
                        nc.sync.dma_start(
                            out=_apd(dst, (w0 + full) * P * dst_stride,
                                     [[dst_stride, rows], [1, WCOLS]]),
                            in_=_ap(tsb[:rows, :], full * WCOLS, [[1, WCOLS]]))

                ncols = min(nwin * P, ncols_total - w0 * P)
                build_windows(src_xT, src_col0 + w0 * P, nwin, ncols, store)
                w0 += nwin

        # ---- phase 1a: full table piece 0 (local compute, no collective)
        build_phase(xTf, 0, SPLIT, t1p[0], TS2)
        # ---- phase 1b: local shard table (self rows / al_dst source)
        build_phase(xTs, 0, SH, ts1, WCOLS)

        # ---- edge phase (shared by both layers) ---------------------------
        gcols = [0]
        ccols = [0]
        for w in range(NW):
            gcols.append(gcols[-1] + 8 * (NCHL[w] + NCHH[w]))
            ccols.append(ccols[-1] + NCH[w])

        # call index base per window (planner order: per window lo then hi)
        PAD0 = os.environ.get("GAT_PAD0") == "1"
        gcnt_reg = None if PAD0 else nc.gpsimd.alloc_register("gcnt")
        call_base = [0]
        for w in range(NW):
            call_base.append(call_base[-1] + math.ceil(NCHL[w] / MAXCK)
                             + math.ceil(NCHH[w] / MAXCK))

        def gather_calls(G, w, which, layer):
            """Issue SWDGE gathers for the lo or hi chunk set of window w."""
            if which == 0:
                nck, c0_out, gc0, tfp = NCHL[w], 0, gcols[w], \
                    (t1p[0] if layer == 0 else tf2[0])
                nrows = cfg.full_piece_rows[0]
                cbase = call_base[w]
            else:
                nck, c0_out, gc0 = NCHH[w], NCHL[w], gcols[w] + 8 * NCHL[w]
                tfp = t1p[1] if layer == 0 else tf2[1]
                nrows = cfg.full_piece_rows[1]
                cbase = call_base[w] + math.ceil(NCHL[w] / MAXCK)
            for j, c0 in enumerate(range(0, nck, MAXCK)):
                cn = min(MAXCK, nck - c0)
                if PAD0:
                    nreg = cn * P
                else:
                    nc.gpsimd.reg_load(
                        gcnt_reg, cnts_sb[0:1, cbase + j:cbase + j + 1])
                    nreg = gcnt_reg
                nc.gpsimd.dma_gather(
                    out_ap=_ap(G[:], (c0_out + c0) * TS2,
                               [[TS2, cn], [1, TS2]]),
                    in_ap=_apd(tfp[:], 0, [[TS2, nrows], [1, TS2]]),
                    idxs_ap=gidx_sb[:, gc0 + 8 * c0:gc0 + 8 * (c0 + cn)],
                    num_idxs=cn * P, num_idxs_reg=nreg,
                    elem_size=TS2, elem_step=TS2, queue_num=next_q())

        tiles = {}

        def prep(w, layer):
            pz = cfg.win_piece(w)
            rows = min(P, SH - w * P)
            nch = NCH[w]
            G = eps.tile([P, NCHMAX * TS2], bf16, tag="G",
                         name=f"G_{layer}_{w}")
            L = eps.tile([P, WCOLS], bf16, tag="L", name=f"L_{layer}_{w}")
            tiles[w] = (G, L)
            # local rows: self h + logits + al_dst
            if layer == 0:
                nc.sync.dma_start(out=L[:rows, :],
                                  in_=ts1[w * P:w * P + rows, :])
            else:
                r_lo = w * P - cfg.piece_r0[pz]
                nc.sync.dma_start(out=L[:rows, :],
                                  in_=ts2[pz][r_lo:r_lo + rows, :WCOLS])
            # self chunk payload: copy local rows into G's last chunk
            nc.scalar.copy(
                out=_ap(G[:rows, :], (nch - 1) * TS2, [[1, WCOLS]]),
                in_=L[:rows, :])
            gather_calls(G, w, 0, layer)

        def body(w, layer):
            rows = min(P, SH - w * P)
            nch = NCH[w]
            G, L = tiles.pop(w)
            gather_calls(G, w, 1, layer)

            # one-hot ST[e, d] = (offs[e, c] == d), all chunks in one op.
            # offs is column-duplicated x2 so every operand's innermost dim
            # is stride-1 (DVE 2x perf mode).
            ST = wk.tile([P, NCHMAX * P], bf16, tag="ST")
            nc.vector.tensor_tensor(
                out=_ap(ST[:], 0, [[P, nch], [2, P // 2], [1, 2]]),
                in0=_ap(offs_sb[:], 2 * ccols[w],
                        [[2, nch], [0, P // 2], [1, 2]]),
                in1=_ap(iota_bf[:], 0, [[0, nch], [2, P // 2], [1, 2]]),
                op=mybir.AluOpType.is_equal)

            # STs = per-chunk transposes of ST (PE), copied out ACT/DVE halves
            STs = wk.tile([P, NCHMAX * P], bf16, tag="STs")
            HB = (NCHMAX + 1) // 2
            for h0 in range(0, nch, HB):
                hn = min(HB, nch - h0)
                STt = psum.tile([P, HB * P], bf16, tag="STt")
                for c in range(hn):
                    nc.tensor.transpose(
                        out=STt[:, c * P:(c + 1) * P],
                        in_=ST[:, (h0 + c) * P:(h0 + c + 1) * P],
                        identity=ident_bf[:, :])
                nc.scalar.copy(out=STs[:, h0 * P:(h0 + hn) * P],
                               in_=STt[:, :hn * P])

            # per-chunk al_dst on PE: agg[ALD] = STs_c^T @ L's al_dst cols
            agg = psum.tile([P, D1 + 4 + 4 * NCHMAX], f32, tag="agg")
            ALD0 = D1 + 4
            for c in range(nch):
                nc.tensor.matmul(
                    out=agg[:, ALD0 + c * 4:ALD0 + (c + 1) * 4],
                    lhsT=STs[:, c * P:(c + 1) * P],
                    rhs=L[:, 260:264], start=True, stop=True)

            # s = al_src + al_dst (small DVE add), leaky relu, exp (ACT)
            score = wk.tile([P, NCHMAX * 4], f32, tag="score")
            nc.vector.tensor_tensor(
                out=score[:, :nch * 4],
                in0=_ap(G[:], 256, [[TS2, nch], [1, 4]]),
                in1=agg[:, ALD0:ALD0 + nch * 4],
                op=mybir.AluOpType.add)
            nc.vector.scalar_tensor_tensor(
                out=score[:, :nch * 4],
                in0=score[:, :nch * 4],
                scalar=0.2,
                in1=score[:, :nch * 4],
                op0=mybir.AluOpType.mult, op1=mybir.AluOpType.max)
            exb = wk.tile([P, NCHMAX * 4], bf16, tag="exb")
            nc.scalar.activation(
                out=exb[:, :nch * 4],
                in_=score[:, :nch * 4],
                func=mybir.ActivationFunctionType.Exp)

            # exh = [ex*h | ex] per chunk (260-col stride). h columns are
            # head-interleaved (col 4c+h), so the head dim is the innermost
            # stride-1 dim for every operand (DVE 2x perf mode).
            exh = wk.tile([P, NCHMAX * (D1 + 4)], bf16, tag="exh")
            nc.vector.tensor_tensor(
                out=_ap(exh[:], 0, [[D1 + 4, nch], [H, C], [1, H]]),
                in0=_ap(G[:], 0, [[TS2, nch], [H, C], [1, H]]),
                in1=_ap(exb[:], 0, [[4, nch], [0, C], [1, H]]),
                op=mybir.AluOpType.mult)
            nc.scalar.copy(
                out=_ap(exh[:], D1, [[D1 + 4, nch], [1, 4]]),
                in_=_ap(exb[:], 0, [[4, nch], [1, 4]]))

            # segment sum via PE: agg[d, 0:260] += ST_c^T @ exh_c
            for cchunk in range(nch):
                nc.tensor.matmul(
                    out=agg[:, :D1 + 4],
                    lhsT=ST[:, cchunk * P:(cchunk + 1) * P],
                    rhs=_ap(exh[:], cchunk * (D1 + 4), [[1, D1 + 4]]),
                    start=(cchunk == 0), stop=(cchunk == nch - 1))

            # normalize: act = relu(agg) * recip(den)
            den = wk.tile([P, 4], f32, tag="den")
            nc.vector.tensor_scalar_max(out=den[:], in0=agg[:, D1:D1 + 4],
                                        scalar1=1e-30)
            rec = wk.tile([P, 4], f32, tag="rec")
            nc.vector.reciprocal(out=rec[:], in_=den[:])
            act = wk.tile([P, D1], f32, tag="act")
            nc.vector.scalar_tensor_tensor(
                out=act[:rows, :],
                in0=agg[:rows, :D1],
                scalar=0.0,
                in1=_ap(rec[:rows, :], 0, [[0, C], [1, H]]),
                op0=mybir.AluOpType.max, op1=mybir.AluOpType.mult)

            if layer == 0:
                pz = cfg.win_piece(w)
                r_lo = w * P - cfg.piece_r0[pz]
                abf = wk.tile([P, D1], bf16, tag="abf")
                nc.scalar.copy(out=abf[:rows, :], in_=act[:rows, :])
                tp = psum.tile([P, D1], bf16, tag="tp")
                for k in range(k2_tiles):
                    nc.tensor.transpose(
                        out=tp[:, k * P:k * P + rows],
                        in_=abf[:rows, k * P:(k + 1) * P],
                        identity=ident_bf[:rows, :rows])
                xT2 = wk.tile([P, D1], bf16, tag="xT2")
                nc.scalar.copy(out=xT2[:, :], in_=tp[:, :])
                t2p = psum.tile([P, D1 + 8], f32, tag="t2p")
                for k in range(k2_tiles):
                    nc.tensor.matmul(
                        out=t2p[:rows, :],
                        lhsT=xT2[:, k * P:k * P + rows],
                        rhs=w2e_sb[k][:],
                        start=(k == 0), stop=(k == k2_tiles - 1))
                tsb = wk.tile([P, WCOLS], bf16, tag="tsb")
                nc.scalar.copy(out=tsb[:rows, :132], in_=t2p[:rows, :132])
                nc.vector.tensor_copy(out=tsb[:rows, 132:WCOLS],
                                      in_=t2p[:rows, 132:WCOLS])
                nc.sync.dma_start(
                    out=ts2[pz][r_lo:r_lo + rows, :WCOLS],
                    in_=tsb[:rows, :WCOLS])
                if w + 1 in cfg.piece_w0[1:].tolist():
                    p = cfg.win_piece(w)
                    nc.gpsimd.collective_compute(
                        "AllGather", mybir.AluOpType.bypass,
                        replica_groups=groups,
                        ins=[ts2[p][:, :]],
                        outs=[_apd(tf2[p], 0,
                                   [[TS2, cfg.full_piece_rows[p]], [1, AGC]])])
            else:
                nc.sync.dma_start(out=out_d[w * P:w * P + rows, :],
                                  in_=act[:rows, :])

        def edge_phase(layer):
            for w in range(min(PRE, NW)):
                prep(w, layer)
            for w in range(NW):
                body(w, layer)
                if w + PRE < NW:
                    prep(w + PRE, layer)

        # issue early layer-0 preps (lo gathers run during piece-1 build)
        for w in range(min(PRE, NW)):
            prep(w, 0)
        # ---- phase 1c: full table piece 1
        build_phase(xTf, SPLIT, N - SPLIT, t1p[1], TS2)

        # ---- edge phases
        for w in range(NW):
            body(w, 0)
            if w + PRE < NW:
                prep(w + PRE, 0)
        tiles.clear()
        edge_phase(1)

    nc.compile()
    return nc


def _make_inputs(cfg, plan, per_core, pnode, x, W1, a1s, a1d, W2, a2s, a2d):
    import ml_dtypes
    bf = ml_dtypes.bfloat16
    w1e = _pack_wext(cfg, np.asarray(W1, np.float32),
                     np.asarray(a1s, np.float32),
                     np.asarray(a1d, np.float32)).astype(bf)
    w2e = _pack_wext(cfg, np.asarray(W2, np.float32),
                     np.asarray(a2s, np.float32),
                     np.asarray(a2d, np.float32), row_ilv=True).astype(bf)
    x = np.asarray(x, np.float32)
    xbf = x.astype(bf)
    node_of_p = np.argsort(pnode)
    # piece-major full xT (same for all cores); pseudo-node j holds
    # x[node_of_p[j]]
    frow = cfg.full_row(np.arange(cfg.n_nodes))
    xTf = np.empty((cfg.in_dim, cfg.n_nodes), dtype=bf)
    xTf[:, frow] = xbf[node_of_p].T
    in_maps = []
    for c in range(cfg.n_cores):
        xs = xbf[node_of_p[c * cfg.shard:(c + 1) * cfg.shard]].T.copy()
        in_maps.append(dict(
            xTf=xTf, xTs=xs, w1e=w1e, w2e=w2e,
            gidx=per_core[c]["gidx"], offs=per_core[c]["offs"],
            counts=per_core[c]["counts"]))
    return in_maps


def _ensure_ntff_hook():
    import types
    try:
        from antenv.axon_hooks import get_axon_ntff_profile_hook  # noqa: F401
        return
    except ImportError:
        pass
    import antenv
    mod = types.ModuleType("antenv.axon_hooks")
    _h = [None]
    mod.set_axon_ntff_profile_hook = lambda h: _h.__setitem__(0, h)
    mod.get_axon_ntff_profile_hook = lambda: _h[0]
    sys.modules["antenv.axon_hooks"] = mod
    antenv.axon_hooks = mod
    try:
        from trn_agent_boot.trn_boot import _ntff_profile_via_ctypes
        mod.set_axon_ntff_profile_hook(
            _ntff_profile_via_ctypes("/opt/axon/libaxon_pjrt.so"))
    except Exception:
        pass


def run(cfg, inputs, trace=False):
    from concourse.bass_utils import run_bass_kernel_spmd

    if trace:
        _ensure_ntff_hook()

    ei = np.asarray(inputs["edge_index"])
    pnode = _balance_nodes(cfg, ei)
    plan, per_core = _plan_edges(cfg, ei, pnode)
    nc = build_program(cfg, plan)
    in_maps = _make_inputs(cfg, plan, per_core, pnode, inputs["x"],
                           inputs["W1"], inputs["a1_src"], inputs["a1_dst"],
                           inputs["W2"], inputs["a2_src"], inputs["a2_dst"])
    b1 = np.asarray(inputs["b1"], np.float32)
    b2 = np.asarray(inputs["b2"], np.float32)
    assert not (np.any(b1) or np.any(b2)), "nonzero biases not supported"
    res = run_bass_kernel_spmd(nc, in_maps, list(range(cfg.n_cores)),
                               trace=trace)
    out_dev = np.concatenate(
        [res.results[c]["out"] for c in range(cfg.n_cores)], axis=0)
    # undo the head interleave (cols) and the balance permutation (rows)
    out = np.empty_like(out_dev)
    node_of_p = np.argsort(pnode)
    out[np.ix_(node_of_p, _ilv(cfg))] = out_dev
    return out, res


def kernel(**inputs) -> np.ndarray:
    cfg = Cfg()
    assert inputs["x"].shape == (cfg.n_nodes, cfg.in_dim)
    out, _ = run(cfg, inputs, trace=False)
    return out.astype(np.float32)
